# revision 1
# baseline (speedup 1.0000x reference)
"""GNN message-passing + pooling kernel for 8 Trainium2 NeuronCores.

Strategy (per the sharding hint):
  - Host: sort edges by dst, partition the 50k nodes into 8 contiguous
    ranges of 6250; each core gets the edges targeting its node range
    (disjoint scatter -> no cross-core reduction needed).
  - Host gathers x[dst], x[src], edge_attr into a transposed bf16
    [320, E_pad] tensor per core (edges grouped into 481-node scatter
    windows, padded to a uniform chunk count so the device program is
    identical across cores).
  - Device (per core): 4-layer message MLP in transposed-activation
    layout (weights stationary on the PE array, edges streaming, f32
    PSUM accumulation), scatter-add via one-hot matmuls (one-hot built
    on-device with iota + is_equal against per-edge local dst), node MLP
    over the core's 6250 nodes, and per-graph sum-pooling via a one-hot
    pooling matmul.  Output: [32, 128] partial per-graph sums.
  - Host: sum the 8 partials, divide by per-graph node counts, apply the
    final [128, 16] linear.
"""

import sys

if "/opt/trn_rl_repo" not in sys.path:
    sys.path.insert(0, "/opt/trn_rl_repo")

import numpy as np
import ml_dtypes

BF16 = ml_dtypes.bfloat16

# Problem dims
N_NODES = 50000
N_EDGES = 800000
NF = 128          # node feature dim
EF = 64           # edge feature dim
MSGD = 128        # message dim
HID = 300         # MLP hidden
G = 32            # graphs
NCORES = 8

# Tiling config
NPC = N_NODES // NCORES   # 6250 nodes per core
NW = 481                  # nodes per scatter window
W = 13                    # windows per core (13*481 = 6253 >= 6250)
ST = 512                  # edge supertile (free dim per matmul)
NP2 = 6656                # padded nodes per core for node MLP (13*512)
NT = NP2 // ST            # node supertiles
NCHK = NP2 // 128         # node chunks for pooling

TRACE = False             # set True from test harness to profile core 0
LAST_EXEC_NS = None

_BUILD_CACHE = {}


def _chunks(total, step=128):
    return [(o, min(step, total - o)) for o in range(0, total, step)]


def _build_nc(C):
    """Build the (single) SPMD Bass program. C = 128-edge chunks per window."""
    import concourse.bacc as bacc
    import concourse.tile as tile
    from concourse import mybir
    from contextlib import ExitStack

    f32 = mybir.dt.float32
    bf16 = mybir.dt.bfloat16
    AF = mybir.ActivationFunctionType
    OP = mybir.AluOpType

    E_pad = W * C * 128
    NCHUNKS = W * C

    nc = bacc.Bacc("TRN2", target_bir_lowering=False, debug=False,
                   num_devices=NCORES)

    # --- DRAM I/O ---
    d_msg_inT = nc.dram_tensor("msg_inT", [2 * NF + EF, E_pad], bf16,
                               kind="ExternalInput")
    d_dstloc = nc.dram_tensor("dstloc", [128, NCHUNKS], f32,
                              kind="ExternalInput")
    d_xT = nc.dram_tensor("xT", [NF, NP2], bf16, kind="ExternalInput")
    d_pmat = nc.dram_tensor("pmat", [128, NCHK * G], bf16,
                            kind="ExternalInput")
    d_mW = [nc.dram_tensor(f"mW{i}", s, bf16, kind="ExternalInput")
            for i, s in enumerate([[2 * NF + EF, HID], [HID, HID], [HID, HID],
                                   [HID, MSGD]], start=1)]
    d_mb = [nc.dram_tensor(f"mb{i}", [HID, 1], f32, kind="ExternalInput")
            for i in range(1, 4)]
    d_mb4r = nc.dram_tensor("mb4r", [1, MSGD], bf16, kind="ExternalInput")
    d_nW = [nc.dram_tensor(f"nW{i}", s, bf16, kind="ExternalInput")
            for i, s in enumerate([[NF + MSGD, HID], [HID, HID], [HID, HID],
                                   [HID, NF]], start=1)]
    d_nb = [nc.dram_tensor(f"nb{i}", [HID, 1], f32, kind="ExternalInput")
            for i in range(1, 4)]
    d_nb4r = nc.dram_tensor("nb4r", [1, NF], bf16, kind="ExternalInput")
    d_out = nc.dram_tensor("partial", [G, NF], f32, kind="ExternalOutput")

    with tile.TileContext(nc) as tc, ExitStack() as ctx:
        wpool = ctx.enter_context(tc.tile_pool(name="w", bufs=1))
        apool = ctx.enter_context(tc.tile_pool(name="agg", bufs=1))
        inpool = ctx.enter_context(tc.tile_pool(name="in", bufs=4))
        hpool = ctx.enter_context(tc.tile_pool(name="h", bufs=2))
        mpool = ctx.enter_context(tc.tile_pool(name="m", bufs=6))
        spool = ctx.enter_context(tc.tile_pool(name="s", bufs=6))
        mm_psum = ctx.enter_context(
            tc.tile_pool(name="mmp", bufs=6, space="PSUM"))
        acc_psum = ctx.enter_context(
            tc.tile_pool(name="accp", bufs=2, space="PSUM"))

        def load_w(dram, K, N, dt, name):
            tiles = []
            for i, (k0, kk) in enumerate(_chunks(K)):
                t = wpool.tile([kk, N], dt, tag=f"{name}{i}")
                nc.sync.dma_start(t[:, :], dram[k0:k0 + kk, :])
                tiles.append(t)
            return tiles

        mW = [load_w(d_mW[0], 2 * NF + EF, HID, bf16, "mW1"),
              load_w(d_mW[1], HID, HID, bf16, "mW2"),
              load_w(d_mW[2], HID, HID, bf16, "mW3"),
              load_w(d_mW[3], HID, MSGD, bf16, "mW4")]
        mb = [load_w(d_mb[i], HID, 1, f32, f"mb{i + 1}") for i in range(3)]
        nW = [load_w(d_nW[0], NF + MSGD, HID, bf16, "nW1"),
              load_w(d_nW[1], HID, HID, bf16, "nW2"),
              load_w(d_nW[2], HID, HID, bf16, "nW3"),
              load_w(d_nW[3], HID, NF, bf16, "nW4")]
        nb = [load_w(d_nb[i], HID, 1, f32, f"nb{i + 1}") for i in range(3)]
        mb4r = wpool.tile([1, MSGD], bf16, tag="mb4r")
        nc.sync.dma_start(mb4r[:, :], d_mb4r[:, :])
        nb4r = wpool.tile([1, NF], bf16, tag="nb4r")
        nc.sync.dma_start(nb4r[:, :], d_nb4r[:, :])

        dstloc = wpool.tile([128, NCHUNKS], f32, tag="dstloc")
        nc.sync.dma_start(dstloc[:, :], d_dstloc[:, :])
        xT = wpool.tile([NF, NP2], bf16, tag="xT")
        nc.sync.dma_start(xT[:, :], d_xT[:, :])
        pmat = wpool.tile([128, NCHK * G], bf16, tag="pmat")
        nc.sync.dma_start(pmat[:, :], d_pmat[:, :])

        iota = wpool.tile([128, NW], f32, tag="iota")
        nc.gpsimd.iota(iota[:, :], pattern=[[1, NW]], base=0,
                       channel_multiplier=0,
                       allow_small_or_imprecise_dtypes=True)
        ones = wpool.tile([1, ST], bf16, tag="ones")
        nc.gpsimd.memset(ones[:, :], 1.0)

        aggrT = apool.tile([NF, NP2], bf16, tag="aggrT")
        # scatter windows cover cols [0, W*NW); zero the tail
        nc.gpsimd.memset(aggrT[:, W * NW:NP2], 0.0)
        pooled = apool.tile([G, NF], f32, tag="pooled")
        nc.vector.memset(pooled[:, :], 0.0)

        HCH = _chunks(HID)       # [(0,128),(128,128),(256,44)]
        KIN = _chunks(2 * NF + EF)  # [(0,128),(128,128),(256,64)]

        def mlp_front(rhs_l1, w123, b123, tagp):
            """Layers 1-3 (transposed activations). rhs_l1: list of
            (tile, kk) K-chunks for layer 1. Returns h3 tiles."""
            h_prev = None
            for layer in range(3):
                if layer == 0:
                    ksrc = rhs_l1
                else:
                    ksrc = [(h_prev[i], kk) for i, (_, kk) in enumerate(HCH)]
                h_cur = []
                for m, (m0, mm) in enumerate(HCH):
                    p = mm_psum.tile([128, ST], mybir.dt.float32, tag="mmp")
                    for k, (kt, kk) in enumerate(ksrc):
                        nc.tensor.matmul(
                            p[:mm, :], w123[layer][k][:, m0:m0 + mm],
                            kt[:kk, :] if layer == 0 else kt[:kk, :],
                            start=(k == 0), stop=(k == len(ksrc) - 1))
                    ht = hpool.tile([128, ST], bf16, tag=f"{tagp}h{layer}_{m}")
                    if layer == 1:
                        nc.vector.tensor_scalar(
                            ht[:mm, :], p[:mm, :], b123[layer][m][:mm, :], 0.0,
                            op0=OP.add, op1=OP.max)
                    else:
                        nc.scalar.activation(ht[:mm, :], p[:mm, :], AF.Relu,
                                             bias=b123[layer][m][:mm, :])
                    h_cur.append(ht)
                h_prev = h_cur
            return h_prev

        # ---- edge phase ----
        for w in range(W):
            accp = acc_psum.tile([128, NW], mybir.dt.float32, tag="acc")
            for g in range(C // 4):
                base = (w * C + g * 4) * 128
                in_t = []
                for i, (k0, kk) in enumerate(KIN):
                    t = inpool.tile([kk, ST], bf16, tag=f"in{i}")
                    nc.sync.dma_start(t[:, :],
                                      d_msg_inT[k0:k0 + kk, base:base + ST])
                    in_t.append((t, kk))
                h3 = mlp_front(in_t, mW, mb, "e")
                for e in range(4):
                    cidx = w * C + g * 4 + e
                    mp = mm_psum.tile([128, MSGD], mybir.dt.float32, tag="mmp")
                    for k, (k0, kk) in enumerate(HCH):
                        nc.tensor.matmul(
                            mp[:, :], h3[k][:kk, e * 128:(e + 1) * 128],
                            mW[3][k][:, :], start=(k == 0), stop=False)
                    nc.tensor.matmul(mp[:, :], ones[:1, :128], mb4r[:1, :],
                                     start=False, stop=True)
                    msgt = mpool.tile([128, MSGD], bf16, tag="msg")
                    nc.scalar.activation(msgt[:, :], mp[:, :], AF.Copy)
                    st = spool.tile([128, NW], bf16, tag="S")
                    nc.vector.tensor_scalar(
                        st[:, :], iota[:, :], dstloc[:, cidx:cidx + 1], None,
                        op0=OP.is_equal)
                    nc.tensor.matmul(accp[:, :], msgt[:, :], st[:, :],
                                     start=(g == 0 and e == 0),
                                     stop=(g == C // 4 - 1 and e == 3),
                                     skip_group_check=True)
            nc.vector.tensor_copy(aggrT[:, w * NW:(w + 1) * NW], accp[:, :])

        # ---- node phase ----
        for t in range(NT):
            rhs = [(xT[:, t * ST:(t + 1) * ST], NF),
                   (aggrT[:, t * ST:(t + 1) * ST], MSGD)]
            # layer 1 K-chunks come from two resident tiles (slices)
            h_prev = None
            for layer in range(3):
                ksrc = rhs if layer == 0 else [
                    (h_prev[i][:kk, :], kk) for i, (_, kk) in enumerate(HCH)]
                h_cur = []
                for m, (m0, mm) in enumerate(HCH):
                    p = mm_psum.tile([128, ST], mybir.dt.float32, tag="mmp")
                    for k, (kt, kk) in enumerate(ksrc):
                        nc.tensor.matmul(p[:mm, :], nW[layer][k][:, m0:m0 + mm],
                                         kt, start=(k == 0),
                                         stop=(k == len(ksrc) - 1))
                    ht = hpool.tile([128, ST], bf16, tag=f"nh{layer}_{m}")
                    if layer == 1:
                        nc.vector.tensor_scalar(
                            ht[:mm, :], p[:mm, :], nb[layer][m][:mm, :], 0.0,
                            op0=OP.add, op1=OP.max)
                    else:
                        nc.scalar.activation(ht[:mm, :], p[:mm, :], AF.Relu,
                                             bias=nb[layer][m][:mm, :])
                    h_cur.append(ht)
                h_prev = h_cur
            for e in range(4):
                tch = t * 4 + e
                np_ = mm_psum.tile([128, NF], mybir.dt.float32, tag="mmp")
                for k, (k0, kk) in enumerate(HCH):
                    nc.tensor.matmul(np_[:, :],
                                     h_prev[k][:kk, e * 128:(e + 1) * 128],
                                     nW[3][k][:, :], start=(k == 0), stop=False)
                nc.tensor.matmul(np_[:, :], ones[:1, :128], nb4r[:1, :],
                                 start=False, stop=True)
                no = mpool.tile([128, NF], bf16, tag="msg")
                nc.scalar.activation(no[:, :], np_[:, :], AF.Copy)
                pp = acc_psum.tile([G, NF], mybir.dt.float32, tag="acc")
                nc.tensor.matmul(pp[:, :], pmat[:, tch * G:(tch + 1) * G],
                                 no[:, :], start=True, stop=True)
                nc.vector.tensor_add(pooled[:, :], pooled[:, :], pp[:, :])

        nc.sync.dma_start(d_out[:, :], pooled[:, :])

    nc.compile()
    return nc


def _prep_inputs(x, edge_index, edge_attr, batch, weights, C):
    """Host-side shard/gather/pad. Returns per-core in_maps."""
    E_pad = W * C * 128
    src = np.asarray(edge_index[0], np.int64)
    dst = np.asarray(edge_index[1], np.int64)

    order = np.argsort(dst, kind="stable")
    dsts = dst[order]
    srcs = src[order]

    xT = np.ascontiguousarray(np.asarray(x, np.float32).astype(BF16).T)
    eaT = np.ascontiguousarray(np.asarray(edge_attr, np.float32).astype(BF16).T)
    batch = np.asarray(batch, np.int64)

    bounds = np.searchsorted(dsts, np.arange(0, N_NODES + 1, NPC))

    wcommon = {}
    for i in range(1, 5):
        wcommon[f"mW{i}"] = np.ascontiguousarray(
            weights[f"mW{i}"].astype(BF16))
        wcommon[f"nW{i}"] = np.ascontiguousarray(
            weights[f"nW{i}"].astype(BF16))
    for i in range(1, 4):
        wcommon[f"mb{i}"] = np.ascontiguousarray(
            weights[f"mb{i}"].reshape(HID, 1).astype(np.float32))
        wcommon[f"nb{i}"] = np.ascontiguousarray(
            weights[f"nb{i}"].reshape(HID, 1).astype(np.float32))
    wcommon["mb4r"] = np.ascontiguousarray(
        weights["mb4"].reshape(1, MSGD).astype(BF16))
    wcommon["nb4r"] = np.ascontiguousarray(
        weights["nb4"].reshape(1, NF).astype(BF16))

    garange = np.arange(G)
    in_maps = []
    for k in range(NCORES):
        sl = slice(int(bounds[k]), int(bounds[k + 1]))
        eidx = order[sl]
        dloc = dsts[sl] - k * NPC
        srck = srcs[sl]
        win = dloc // NW
        cnt = np.bincount(win, minlength=W)

        starts = np.repeat(np.arange(W) * C * 128, cnt)
        within = np.arange(len(dloc)) - np.repeat(np.cumsum(cnt) - cnt, cnt)
        pos = starts + within

        msg_inT = np.zeros((2 * NF + EF, E_pad), BF16)
        msg_inT[0:NF, pos] = xT[:, k * NPC + dloc]
        msg_inT[NF:2 * NF, pos] = xT[:, srck]
        msg_inT[2 * NF:, pos] = eaT[:, eidx]

        dl = np.full(E_pad, -1.0, np.float32)
        dl[pos] = (dloc - win * NW).astype(np.float32)
        dstloc = np.ascontiguousarray(dl.reshape(E_pad // 128, 128).T)

        xTn = np.zeros((NF, NP2), BF16)
        xTn[:, :NPC] = xT[:, k * NPC:(k + 1) * NPC]

        bl = np.full(NP2, -1, np.int64)
        bl[:NPC] = batch[k * NPC:(k + 1) * NPC]
        P = (bl[:, None] == garange[None, :]).astype(BF16)
        pmat = np.ascontiguousarray(
            P.reshape(NCHK, 128, G).transpose(1, 0, 2).reshape(128, NCHK * G))

        in_map = dict(wcommon)
        in_map.update(msg_inT=msg_inT, dstloc=dstloc, xT=xTn, pmat=pmat)
        in_maps.append(in_map)
    return in_maps


def kernel(**inputs):
    global LAST_EXEC_NS
    from concourse.bass_utils import run_bass_kernel_spmd

    x = np.asarray(inputs["x"], np.float32)
    edge_index = np.asarray(inputs["edge_index"])
    edge_attr = np.asarray(inputs["edge_attr"], np.float32)
    batch = np.asarray(inputs["batch"])

    # chunk count per window from the actual data (uniform across cores)
    dst = np.asarray(edge_index[1], np.int64)
    dloc_all = dst % NPC
    core_all = dst // NPC
    win_all = dloc_all // NW
    cnt = np.bincount(core_all * W + win_all, minlength=NCORES * W)
    C = int(np.ceil(cnt.max() / 128.0))
    C = max(4, int(np.ceil(C / 4.0)) * 4)

    key = C
    if key not in _BUILD_CACHE:
        _BUILD_CACHE[key] = _build_nc(C)
    nc = _BUILD_CACHE[key]

    in_maps = _prep_inputs(x, edge_index, edge_attr, batch, inputs, C)

    res = run_bass_kernel_spmd(nc, in_maps, list(range(NCORES)), trace=TRACE)
    LAST_EXEC_NS = res.exec_time_ns

    total = np.zeros((G, NF), np.float64)
    for r in res.results:
        total += np.asarray(r["partial"], np.float64)

    counts = np.bincount(np.asarray(batch, np.int64), minlength=G)
    pooled = (total / np.maximum(counts, 1)[:, None]).astype(np.float32)
    out = pooled @ np.asarray(inputs["linW"], np.float32) + np.asarray(
        inputs["linb"], np.float32)
    return out.astype(np.float32)

